# revision 1
# baseline (speedup 1.0000x reference)
"""Distributed ALiBi causal attention for 8 TRN2 NeuronCores.

Sharding: core c = (b, hg) with b = c // 4 (batch), hg = c % 4 (group of 4
heads = 256 of the 1024 model dims).  Each core:
  1. computes q/k/v projections for its 256-dim column slice of Wq/Wk/Wv
     (transposed layout: qT/kT = W.T @ x.T so the head dim lands on
     partitions),
  2. runs causal attention for its 4 heads in "scores transposed" form
     (scoresT[j, i] = k_j . q_i): the ALiBi bias reduces to a per-key
     -slope*j term (the +slope*i part is constant per softmax row and
     drops out), applied as the per-partition bias of the exp activation
     (shifted by -ln 64 so sums stay in fp16 range; the shift cancels in
     the softmax ratio); softmax denominators come free from a
     ones-column appended to V,
  3. AllGathers the per-head attention outputs across its 4-core batch
     group (chunked by q-range, fp16, overlapped with the next chunk's
     attention), and computes a disjoint 256-column slice of the final
     out projection (contraction over all 16 heads).
The host only slices inputs and concatenates the 8 output slices.

Matmuls run in fp16 (10-bit mantissa = TF32-level accuracy at full PE
rate); accumulation is always fp32 in PSUM.
"""

import math
import os

import numpy as np

B = 2
T = 2048
C = 1024
H = 16
D = 64
N_CORES = 8
HG = 4          # head groups (cores per batch)
HL = 4          # heads per core
DG = HL * D     # 256 d-dims per core
CI = C // 128   # 8 contraction chunks of 128
TB = T // 128   # 16 row blocks of 128
QC = T // 512   # 4 q chunks of 512
KB = T // 128   # 16 key blocks of 128
VROW = D + 1    # v columns per head incl. ones column
LN_SHIFT = float(np.log(64.0))

REPLICA_GROUPS = [[0, 1, 2, 3], [4, 5, 6, 7]]

_COMPILED = None
last_exec_time_ns = None
last_trace_path = None


def _alibi_slopes(n_heads: int) -> np.ndarray:
    def pow2_slopes(n):
        start = 2 ** (-(2 ** (-(math.log2(n) - 3))))
        return [start * start**i for i in range(n)]

    if math.log2(n_heads).is_integer():
        s = pow2_slopes(n_heads)
    else:
        c = 2 ** math.floor(math.log2(n_heads))
        s = pow2_slopes(c)
        s.extend(pow2_slopes(2 * c)[0::2][: n_heads - c])
    return np.array(s, dtype=np.float32)


def _build():
    import concourse.mybir as mybir
    import concourse.tile as tile
    from concourse.alu_op_type import AluOpType
    from concourse.bacc import Bacc
    from contextlib import ExitStack

    F32 = mybir.dt.float32
    F16 = mybir.dt.float16
    ACT = mybir.ActivationFunctionType

    nc = Bacc(None, target_bir_lowering=False, num_devices=N_CORES)

    xT_ext = nc.declare_dram_parameter("xT", [C, T], F16, isOutput=False)
    wq_ext = nc.declare_dram_parameter("wq", [C, DG], F16, isOutput=False)
    wk_ext = nc.declare_dram_parameter("wk", [C, DG], F16, isOutput=False)
    wv_ext = nc.declare_dram_parameter("wv", [C, HL * D], F16, isOutput=False)
    wo_ext = nc.declare_dram_parameter("wo", [C, DG], F16, isOutput=False)
    bq_ext = nc.declare_dram_parameter("bq2", [2, 128], F32, isOutput=False)
    bk_ext = nc.declare_dram_parameter("bk2", [2, 128], F32, isOutput=False)
    # expb[p, h*TB+tb] = exp(-slope_h * (128*tb+p) - ln 64): the ALiBi bias
    # as a per-key multiplicative prescale of V (incl. its ones column), so
    # the exp activation needs no per-head bias.
    expb_ext = nc.declare_dram_parameter("expb", [128, HL * TB], F32, isOutput=False)
    tri_ext = nc.declare_dram_parameter("tri", [128, 128], F16, isOutput=False)
    out_ext = nc.declare_dram_parameter("out", [T, DG], F32, isOutput=True)

    with tile.TileContext(nc) as tc, ExitStack() as ctx:
        persist = ctx.enter_context(tc.tile_pool(name="persist", bufs=1))
        wo_sb = persist.tile([128, CI * DG], F16)
        expb_sb = persist.tile([128, HL * TB], F32)
        bq_sb = persist.tile([128, 2], F32)
        bk_sb = persist.tile([128, 2], F32)
        tri_sb = persist.tile([128, 128], F16)

        qkv = ctx.enter_context(tc.tile_pool(name="qkv", bufs=1))
        qT_sb = qkv.tile([128, 2 * T], F16)
        kT_sb = qkv.tile([128, 2 * T], F16)
        v_sb = qkv.tile([128, TB * HL * VROW], F16)
        attn_sb = qkv.tile([128, 2 * T], F16)
        g_sb = qkv.tile([128, CI * T], F16)

        # ---------------- Phase 1: projections -------------------------
        with ExitStack() as p1:
            xw = p1.enter_context(tc.tile_pool(name="xw", bufs=1))
            xT_sb = xw.tile([128, CI * T], F16)
            wq_sb = xw.tile([128, CI * DG], F16)
            wk_sb = xw.tile([128, CI * DG], F16)
            wv_sb = xw.tile([128, CI * HL * D], F16)
            for ci in range(CI):
                nc.sync.dma_start(
                    xT_sb[:, ci * T : (ci + 1) * T],
                    xT_ext[ci * 128 : (ci + 1) * 128, :],
                )
                nc.sync.dma_start(
                    wq_sb[:, ci * DG : (ci + 1) * DG],
                    wq_ext[ci * 128 : (ci + 1) * 128, :],
                )
                nc.sync.dma_start(
                    wk_sb[:, ci * DG : (ci + 1) * DG],
                    wk_ext[ci * 128 : (ci + 1) * 128, :],
                )
                nc.sync.dma_start(
                    wv_sb[:, ci * HL * D : (ci + 1) * HL * D],
                    wv_ext[ci * 128 : (ci + 1) * 128, :],
                )

            nc.sync.dma_start(tri_sb[:], tri_ext[:])
            nc.sync.dma_start(expb_sb[:], expb_ext[:])
            for db in range(2):
                nc.sync.dma_start(bq_sb[:, db : db + 1], bq_ext[db : db + 1, :])
                nc.sync.dma_start(bk_sb[:, db : db + 1], bk_ext[db : db + 1, :])
            for ci in range(CI):
                nc.sync.dma_start(
                    wo_sb[:, ci * DG : (ci + 1) * DG],
                    wo_ext[ci * 128 : (ci + 1) * 128, :],
                )

            pps = p1.enter_context(tc.tile_pool(name="pps", bufs=3, space="PSUM"))

            # qT / kT: [256 d, 2048 t] as 2 partition blocks of 128.
            # kT first in key order, then qT starting with the last q-chunk:
            # attention processes chunks in reverse order.
            def proj_ranges():
                for qc in range(QC):
                    yield wk_sb, kT_sb, bk_sb, 0, qc
                    yield wk_sb, kT_sb, bk_sb, 1, qc
                for qc in reversed(range(QC)):
                    yield wq_sb, qT_sb, bq_sb, 0, qc
                    yield wq_sb, qT_sb, bq_sb, 1, qc

            for w_sb, t_sb, b_sb, db, qc in proj_ranges():
                if True:
                    if True:
                        ps = pps.tile([128, 512], F32, tag="proj")
                        for ci in range(CI):
                            nc.tensor.matmul(
                                ps[:],
                                w_sb[:, ci * DG + db * 128 : ci * DG + db * 128 + 128],
                                xT_sb[:, ci * T + qc * 512 : ci * T + qc * 512 + 512],
                                start=(ci == 0),
                                stop=(ci == CI - 1),
                            )
                        nc.scalar.activation(
                            t_sb[:, db * T + qc * 512 : db * T + qc * 512 + 512],
                            ps[:],
                            ACT.Identity,
                            bias=b_sb[:, db : db + 1],
                        )

            # v in augmented layout [t, 4 heads x (64 dims + expb col)], with
            # every row j prescaled by expb[j,h] = exp(-slope_h*j - ln 64):
            # the ALiBi bias as a multiplicative per-key factor.  The
            # "denominator" column holds expb itself (copied once per head,
            # strided across row blocks); data columns come from the v-proj
            # PSUM via per-head tensor_scalar multiplies by the expb column.
            for h in range(HL):
                nc.vector.tensor_copy(
                    v_sb[:, :].rearrange("p (t x) -> p t x", t=TB)[
                        :, :, h * VROW + D : h * VROW + D + 1
                    ],
                    expb_sb[:, h * TB : (h + 1) * TB].rearrange(
                        "p (t x) -> p t x", t=TB
                    ),
                )
            for tb in range(TB):
                ps = pps.tile([128, HL * D], F32, tag="vproj")
                for ci in range(CI):
                    nc.tensor.matmul(
                        ps[:],
                        xT_sb[:, ci * T + tb * 128 : ci * T + tb * 128 + 128],
                        wv_sb[:, ci * HL * D : (ci + 1) * HL * D],
                        start=(ci == 0),
                        stop=(ci == CI - 1),
                    )
                for h in range(HL):
                    nc.vector.tensor_scalar_mul(
                        v_sb[
                            :,
                            tb * HL * VROW + h * VROW : tb * HL * VROW
                            + h * VROW
                            + D,
                        ],
                        ps[:, h * D : (h + 1) * D],
                        expb_sb[:, h * TB + tb : h * TB + tb + 1],
                    )

        # -------- Phase 2+3+4: attention / AllGather / out-proj --------
        # interleaved per q-chunk so the collective overlaps compute
        dram = ctx.enter_context(tc.tile_pool(name="dram", bufs=1, space="DRAM"))
        # tiny warm-up AllGather: absorbs the inter-core start skew during
        # the projection phase so the first real gather's barrier wait does
        # not stall the attention pipeline
        warm_in = dram.tile([1, 8], F16, tag="warm_in", name="warm_in")
        warm_out = dram.tile([HG, 8], F16, tag="warm_out", name="warm_out")
        nc.sync.dma_start(warm_in[:], tri_ext[0:1, 0:8])
        nc.gpsimd.collective_compute(
            "AllGather",
            mybir.AluOpType.bypass,
            replica_groups=REPLICA_GROUPS,
            ins=[warm_in[:].opt()],
            outs=[warm_out[:].opt()],
        )
        with ExitStack() as p2:
            # PSUM bank budget (8 banks): qk 2x2banks + av 2 + wo 1.
            # One [128, 1024] qk tile holds both heads' scoresT; both QK
            # matmuls gate on the same exp having drained it, so the pair is
            # always co-ready -> adjacent in the static schedule -> runs
            # concurrently on disjoint PE row groups (64x128 tiling mode).
            qk_ps = p2.enter_context(tc.tile_pool(name="qk_ps", bufs=2, space="PSUM"))
            av_ps = p2.enter_context(tc.tile_pool(name="av_ps", bufs=1, space="PSUM"))
            wo_ps = p2.enter_context(tc.tile_pool(name="wo_ps", bufs=1, space="PSUM"))
            expp = p2.enter_context(tc.tile_pool(name="expp", bufs=4))
            nrm = p2.enter_context(tc.tile_pool(name="nrm", bufs=2))
            outp = p2.enter_context(tc.tile_pool(name="outp", bufs=3))

            def emit_wo_tb(tb):
                wp = wo_ps.tile([128, DG], F32, tag="wo", name=f"wp{tb}")
                for ci in range(CI):
                    nc.tensor.matmul(
                        wp[:],
                        g_sb[:, ci * T + tb * 128 : ci * T + tb * 128 + 128],
                        wo_sb[:, ci * DG : (ci + 1) * DG],
                        start=(ci == 0),
                        stop=(ci == CI - 1),
                    )
                ot = outp.tile([128, DG], F32, tag="out", name=f"ot{tb}")
                nc.vector.tensor_copy(ot[:], wp[:])
                nc.sync.dma_start(out_ext[tb * 128 : (tb + 1) * 128, :], ot[:])

            def make_norm(qc, hp, dstrip):
                # normalization + gather for unit (qc, hp); deferred into the
                # next unit's kb loop so the slow DMA->reciprocal chain and
                # the collective never head-of-line-block the PE queue
                def run():
                    # normalization off the PE: reciprocal on DVE,
                    # replication on GPSIMD (partition_broadcast sources at
                    # partition 0 and writes partitions [0, n); head 1's
                    # replica is DMA-shifted up to partitions 64-127)
                    bcs = nrm.tile([128, 512], F16, tag="bcs")
                    bct = nrm.tile([64, 512], F16, tag="bct")
                    for hl in range(2):
                        ra = nrm.tile([1, 512], F32, tag=f"rall{hl}")
                        nc.sync.dma_start(
                            ra[:], dstrip[64:65, hl * 512 : (hl + 1) * 512]
                        )
                        rr32 = nrm.tile([1, 512], F32, tag=f"rr32{hl}")
                        nc.vector.reciprocal_approx_fast(rr32[:], ra[:])
                        rr16 = nrm.tile([1, 512], F16, tag=f"rr16{hl}")
                        with nc.allow_low_precision(reason="fp16 bcast input"):
                            nc.vector.tensor_copy(rr16[:], rr32[:])
                        if hl == 0:
                            nc.gpsimd.partition_broadcast(
                                bcs[0:64, :], rr16[0:1, :]
                            )
                        else:
                            nc.gpsimd.partition_broadcast(bct[:], rr16[0:1, :])
                            nc.gpsimd.dma_start(bcs[64:128, :], bct[:])
                    a_pair = attn_sb[
                        :, hp * T + qc * 512 : hp * T + qc * 512 + 512
                    ]
                    nc.vector.tensor_tensor(a_pair, a_pair, bcs[:], AluOpType.mult)
                    attn_dram = dram.tile(
                        [128, 512], F16, tag=f"ad{qc}_{hp}", name=f"ad{qc}_{hp}"
                    )
                    gathered = dram.tile(
                        [HG * 128, 512], F16, tag=f"gd{qc}_{hp}", name=f"gd{qc}_{hp}"
                    )
                    nc.sync.dma_start(attn_dram[:], a_pair)
                    nc.gpsimd.collective_compute(
                        "AllGather",
                        mybir.AluOpType.bypass,
                        replica_groups=REPLICA_GROUPS,
                        ins=[attn_dram[:].opt()],
                        outs=[gathered[:].opt()],
                    )
                    for g in range(HG):
                        ci = 2 * g + hp
                        nc.sync.dma_start(
                            g_sb[:, ci * T + qc * 512 : ci * T + qc * 512 + 512],
                            gathered[g * 128 : (g + 1) * 128, :],
                        )
                    if hp == 1:
                        # both gathers of chunk qc are now in flight: stage
                        # its out-projection row blocks (drained one further
                        # unit later so the collective has ~2 units of slack)
                        wo_stage.extend(range(4 * qc, 4 * qc + 4))

                return run

            qc_order = [QC - 1 - i for i in range(QC)]
            units = [(qc, hp) for qc in qc_order for hp in range(2)]
            pending_norm = None
            wo_queue = []
            wo_stage = []
            for qc, hp in units:
                nkb = 4 * (qc + 1)
                h0, h1 = 2 * hp, 2 * hp + 1
                dstrip = nrm.tile([65, 2 * 512], F32, tag="dstrip", name="ds")
                q0 = qT_sb[0:64, hp * T + qc * 512 : hp * T + qc * 512 + 512]
                q1 = qT_sb[64:128, hp * T + qc * 512 : hp * T + qc * 512 + 512]
                av0 = av_ps.tile([VROW, 512], F32, tag="av0", name="av0")
                av1 = av_ps.tile([VROW, 512], F32, tag="av1", name="av1")
                LAG = 2
                ets = {}

                def emit_av(kb):
                    r = kb - 4 * qc
                    c0 = 128 * r if r > 0 else 0
                    et = ets.pop(kb)
                    for h, avp, eoff in ((h0, av0, 0), (h1, av1, 512)):
                        nc.tensor.matmul(
                            avp[:, c0:512],
                            v_sb[
                                :,
                                kb * HL * VROW + h * VROW : kb * HL * VROW
                                + (h + 1) * VROW,
                            ],
                            et[:, eoff + c0 : eoff + 512],
                            start=(kb == 0),
                            stop=(kb == nkb - 1),
                        )

                def emit_qk(kb):
                    r = kb - 4 * qc
                    c0 = 128 * r if r > 0 else 0
                    # both heads' scoresT in one 2-bank tile; the two
                    # matmuls ride different PE row groups and run
                    # concurrently
                    qkp = qk_ps.tile([128, 1024], F32, tag="qk", name="qkp")
                    nc.tensor.matmul(
                        qkp[:, c0:512],
                        kT_sb[
                            0:64,
                            hp * T + kb * 128 : hp * T + kb * 128 + 128,
                        ],
                        q0[:, c0:512],
                        start=True,
                        stop=True,
                    )
                    nc.tensor.matmul(
                        qkp[:, 512 + c0 : 1024],
                        kT_sb[
                            64:128,
                            hp * T + kb * 128 : hp * T + kb * 128 + 128,
                        ],
                        q1[:, c0:512],
                        start=True,
                        stop=True,
                    )
                    et = expp.tile([128, 1024], F16, tag="exp", name="e")
                    ets[kb] = et
                    # single bias-free exp over both heads (ALiBi lives in
                    # the V prescale); the [0:c0) strips are
                    # stale-but-finite junk that AV never reads
                    nc.scalar.activation(
                        et[:], qkp[:], ACT.Exp, scale=float(D) ** -0.5
                    )
                    if r >= 0:
                        for eoff in (0, 512):
                            nc.vector.tensor_tensor(
                                et[:, eoff + c0 : eoff + c0 + 128],
                                et[:, eoff + c0 : eoff + c0 + 128],
                                tri_sb[:],
                                AluOpType.mult,
                            )

                # kb processed in pairs: 4 QK matmuls (64x128 tiling mode)
                # then 4 AV matmuls (128x128 mode) per step, halving the
                # PE mode-switch drains
                for kb2 in range(0, nkb + LAG, 2):
                    if kb2 == 2 and pending_norm is not None:
                        pending_norm()
                        pending_norm = None
                    for kb in (kb2, kb2 + 1):
                        if kb < nkb:
                            emit_qk(kb)
                    for kb in (kb2, kb2 + 1):
                        if LAG <= kb < nkb + LAG:
                            emit_av(kb - LAG)

                # denominators + unnormalized numerators (vector only)
                for hl, avp in ((0, av0), (1, av1)):
                    d_sl = dstrip[64:65, hl * 512 : (hl + 1) * 512]
                    a_sl = attn_sb[
                        hl * 64 : hl * 64 + 64,
                        hp * T + qc * 512 : hp * T + qc * 512 + 512,
                    ]
                    nc.vector.tensor_copy(d_sl, avp[D : D + 1, :])
                    nc.vector.tensor_copy(a_sl, avp[0:D, :])

                # out-projection row blocks whose gathers fired >= 2 units
                # ago, then promote the freshly staged ones
                while wo_queue:
                    emit_wo_tb(wo_queue.pop(0))
                wo_queue, wo_stage = wo_stage, wo_queue

                pending_norm = make_norm(qc, hp, dstrip)

            pending_norm()
            for tb in wo_queue + wo_stage:
                emit_wo_tb(tb)

    nc.compile()
    return nc


def _get_compiled():
    global _COMPILED
    if _COMPILED is None:
        _COMPILED = _build()
    return _COMPILED


def _make_in_maps(x, Wq, bq, Wk, bk, Wv, bv, Wo, bo):
    slopes = _alibi_slopes(H)
    # tri[p, f] = 1 where key-offset p <= q-offset f (causal keep region)
    tri = np.triu(np.ones((128, 128), dtype=np.float16))
    in_maps = []
    for c in range(N_CORES):
        b, hg = divmod(c, HG)
        sl = slice(hg * DG, (hg + 1) * DG)
        # expb[p, h*TB+tb] = exp(-slope_h * j - ln 64) at key j = 128*tb + p
        expb = np.empty((128, HL * TB), dtype=np.float32)
        p = np.arange(128, dtype=np.float64)[:, None]
        for h in range(HL):
            s = float(slopes[hg * HL + h])
            tbs = np.arange(TB, dtype=np.float64)[None, :]
            expb[:, h * TB : (h + 1) * TB] = np.exp(
                -s * (128.0 * tbs + p) - LN_SHIFT
            ).astype(np.float32)
        in_maps.append(
            {
                "xT": np.ascontiguousarray(x[b].T).astype(np.float16),
                "wq": np.ascontiguousarray(Wq[:, sl]).astype(np.float16),
                "wk": np.ascontiguousarray(Wk[:, sl]).astype(np.float16),
                "wv": np.ascontiguousarray(Wv[:, sl]).astype(np.float16),
                "wo": np.ascontiguousarray(Wo[:, sl]).astype(np.float16),
                "bq2": np.ascontiguousarray(bq[sl].reshape(2, 128)).astype(np.float32),
                "bk2": np.ascontiguousarray(bk[sl].reshape(2, 128)).astype(np.float32),
                "expb": expb,
                "tri": tri,
            }
        )
    return in_maps


def kernel(x, Wq, bq, Wk, bk, Wv, bv, Wo, bo):
    global last_exec_time_ns, last_trace_path
    x = np.asarray(x, dtype=np.float32)
    Wq = np.asarray(Wq, dtype=np.float32)
    bq = np.asarray(bq, dtype=np.float32)
    Wk = np.asarray(Wk, dtype=np.float32)
    bk = np.asarray(bk, dtype=np.float32)
    Wv = np.asarray(Wv, dtype=np.float32)
    bv = np.asarray(bv, dtype=np.float32)
    Wo = np.asarray(Wo, dtype=np.float32)
    bo = np.asarray(bo, dtype=np.float32)

    from concourse import bass_utils

    trace = bool(os.environ.get("BASS_KERNEL_TRACE"))
    kwargs = {}
    if trace:
        try:
            import sys
            import types

            import antenv

            if "antenv.axon_hooks" not in sys.modules:
                hooks = types.ModuleType("antenv.axon_hooks")
                _h = [None]
                hooks.set_axon_ntff_profile_hook = lambda fn: _h.__setitem__(0, fn)
                hooks.get_axon_ntff_profile_hook = lambda: _h[0]
                sys.modules["antenv.axon_hooks"] = hooks
                antenv.axon_hooks = hooks
                from trn_agent_boot.trn_boot import _ntff_profile_via_ctypes

                hooks.set_axon_ntff_profile_hook(
                    _ntff_profile_via_ctypes("/opt/axon/libaxon_pjrt.so")
                )
            bass_utils.upload_artifacts = lambda tmpdir: "local://" + str(tmpdir)
            kwargs = {"trace": True, "tmpdir": os.environ.get("BASS_KERNEL_TRACE_DIR")}
        except Exception as e:  # pragma: no cover
            print(f"trace setup failed ({e}); running untraced")
            trace = False

    nc = _get_compiled()
    in_maps = _make_in_maps(x, Wq, bq, Wk, bk, Wv, bv, Wo, bo)
    res = bass_utils.run_bass_kernel_spmd(
        nc, in_maps, core_ids=list(range(N_CORES)), **kwargs
    )
    if trace:
        last_exec_time_ns = res.exec_time_ns
        if res.instructions_and_trace is not None:
            last_trace_path = res.instructions_and_trace[1]

    # final-projection bias (incl. the v bias folded through Wo) on host
    bfin = bv @ Wo + bo  # [C]
    out = np.empty((B, T, C), dtype=np.float32)
    for c in range(N_CORES):
        b, hg = divmod(c, HG)
        sl = slice(hg * DG, (hg + 1) * DG)
        out[b, :, sl] = res.results[c]["out"] + bfin[sl]
    return out

